# revision 48
# baseline (speedup 1.0000x reference)
"""Trainium2 Bass kernel for nn_DecoderLayer (self-attn + cross-attn + FFN).

Sharding: 8 cores = 4 batch elements x 2 query-halves. Each core computes
its 512 query rows end-to-end (data-parallel over batch, sequence-parallel
over queries). K/V work is recomputed per core from the full per-batch
sequence -- no collectives needed.

Per-core token permutation: the core's own query rows are moved to the
front of the sequence so a single SPMD program (fixed column ranges) works
for both query-halves. In the permuted order the causal mask becomes: a
local lower-triangle over the first tq keys (same for every core,
generated on-device via affine_select) plus an all-or-nothing block over
the remaining keys (a per-core scalar bias folded into the exp).

Scores are computed in the transposed orientation ST[k, q] (K=64 matmuls
packed two heads per PE row group); exp runs on ScalarE with the softmax
denominator coming free from an extra ones-column in the token-major V.
The head loop is software-pipelined (head h's scores hide head h-1's
exp->AV latency). Self-attention uses a token-major AV (denominators land
per-partition: no broadcast matmul, no partition-shift DMA; the output is
PE-transposed back to feature-major for the O-projection). Cross-attention
keeps the feature-major AV whose reciprocal row is broadcast via a K=1
ones-matmul, because coverage needs it: covT += est * recip/h as one
broadcast bf16 multiply in-place on the dead est tile, bf16 pair sums,
and fp32 accumulation split DVE/GpSimd by key half.

Engine budget: matmuls on PE; exp / projection PSUM drains / layernorm
normalize on ScalarE (a single act table serves Ln/Exp/Copy/Relu/Identity
-- layernorm rstd is exp(-0.5*ln(var+eps)), so no Sqrt table reloads);
layernorm stats, triangle mask multiply, reciprocals and coverage on DVE
with GpSimd taking half the coverage accumulation.

All matmuls run in bf16 (fp32 accumulation in PSUM); layernorm stats,
softmax denominators, residual stream, coverage and outputs stay fp32.
`reps` emits the body N times back-to-back (buffer reuse serializes the
repetitions) so burst-slope timing can divide out the per-dispatch axon
overhead; `nowdma` drops the weight transfers for DMA-cost experiments.
"""

import numpy as np
import ml_dtypes

import concourse.bass as bass
import concourse.mybir as mybir
import concourse.tile as tile
from concourse import bacc
from concourse.bass_utils import run_bass_kernel_spmd
from concourse.masks import make_identity

# problem dims (hardcoded per the grading contract)
B, T, D, H, F = 4, 1024, 1024, 16, 4096
DH = D // H
NCORES = 8
QSH = NCORES // B          # query shards per batch element
TQ = T // QSH              # query rows per core
EPS = 1e-5
NEGMASK = -30000.0         # additive mask value (exp -> exact 0)
P = 128
f32 = mybir.dt.float32
bf16 = mybir.dt.bfloat16
fp8 = mybir.dt.float8e4
DR = mybir.MatmulPerfMode.DoubleRow
AF = mybir.ActivationFunctionType
ALU = mybir.AluOpType

# fp8e4m3 DoubleRow for the dense projections (Q/K/V/O, FFN): weights and
# projection activations quantize to fp8, matmuls contract two 128-row
# tiles per pass. Attention core (scores, exp, AV) stays bf16.
FP8 = False

NCHUNK = 512               # matmul free-dim chunk (one PSUM bank of fp32)


def _dedup_act_loads(nc):
    """All ScalarE funcs used here (Ln/Exp/Copy/Relu) live in one act-func
    set; Bacc's per-instruction chooser still ping-pongs between smaller
    sets, emitting a 1.3us table load per switch. Rewrite every load to the
    common set and drop all but the first."""
    from concourse.hw_specs import get_activation_tables
    tabs = list(get_activation_tables(nc.m.arch).items())
    target = next(i for i, (nm, s) in enumerate(tabs)
                  if nm == "natural_log_exp_and_others")
    tset = tabs[target][1]
    need = set()
    for fn in nc.m.functions:
        for blk in fn.blocks:
            for ins in blk.instructions:
                if isinstance(ins, mybir.InstActivation):
                    need.add(ins.func)
    if not need.issubset(tset):
        return
    first = True
    for fn in nc.m.functions:
        for blk in fn.blocks:
            keep = []
            for ins in blk.instructions:
                if isinstance(ins, mybir.InstLoadActFuncSet):
                    si = getattr(ins, "sync_info", None)
                    has_sync = si is not None and (si.on_wait or si.on_update)
                    if first or has_sync:
                        ins.act_func_set_id = target
                        keep.append(ins)
                        first = False
                    continue
                keep.append(ins)
            blk.instructions[:] = keep


def _build_program(t, tq, d, h, f, flags, reps=1, nowdma=False):
    """Build the single-core SPMD program."""
    dh = 64
    dtt = d // P           # feature tiles
    tt = t // P            # token tiles (kv length)
    qtt = tq // P          # own query tiles
    ft = f // P
    csd = min(NCHUNK, d)
    ncd = d // csd
    csf = min(NCHUNK, f)
    wcs = min(NCHUNK, d)   # weight output-column chunk
    nwc = d // wcs
    bnsub = d // min(512, d)
    kper = 2 if tt % 2 == 0 else 1
    kpairs = tt // kper
    dr = FP8 and dtt % 2 == 0
    adt = fp8 if dr else bf16     # fp8 activations: h3T, OTn, fT only

    # Bacc (not plain Bass): its finalize() runs generate_event_semaphores,
    # which splits multi-wait sync onto EventSemaphore carriers -- this
    # walrus build accepts at most 1 wait per instruction.
    nc = bacc.Bacc()

    # ---- DRAM I/O ----
    x_d = nc.dram_tensor("x", [t, d], f32, kind="ExternalInput")
    xb_d = nc.dram_tensor("xb", [t, d], bf16, kind="ExternalInput")
    ctxT_d = nc.dram_tensor("ctxT", [d, t], bf16, kind="ExternalInput")
    sablk_d = nc.dram_tensor("sa_blk", [P, 1], f32, kind="ExternalInput")
    mcacol_d = nc.dram_tensor("mca_col", [P, tt], f32, kind="ExternalInput")
    w_d = {}
    for pre in ("sa", "ca"):
        for nm in ("q", "k", "v", "o"):
            w_d[f"{pre}_{nm}"] = nc.dram_tensor(
                f"w{nm}_{pre}", [d, d], adt if nm == "o" else bf16,
                kind="ExternalInput")
    w1_d = nc.dram_tensor("w1", [d, f], adt, kind="ExternalInput")
    w2_d = nc.dram_tensor("w2", [f, d], adt, kind="ExternalInput")
    bq_d = {pre: nc.dram_tensor(f"bq_{pre}", [P, dtt], f32,
                                kind="ExternalInput") for pre in ("sa", "ca")}
    bk_d = {pre: nc.dram_tensor(f"bk_{pre}", [P, dtt], f32,
                                kind="ExternalInput") for pre in ("sa", "ca")}
    b1_d = nc.dram_tensor("b1", [P, ft], f32, kind="ExternalInput")
    bv_d = {pre: nc.dram_tensor(f"bv_{pre}", [P, d], f32,
                                kind="ExternalInput") for pre in ("sa", "ca")}
    bo_d = {pre: nc.dram_tensor(f"bo_{pre}", [P, d], f32,
                                kind="ExternalInput") for pre in ("sa", "ca")}
    b2_d = nc.dram_tensor("b2", [P, d], f32, kind="ExternalInput")
    lng_d = [nc.dram_tensor(f"ln{i}_g", [P, d], f32, kind="ExternalInput")
             for i in (1, 2, 3)]
    lnb_d = [nc.dram_tensor(f"ln{i}_b", [P, d], f32, kind="ExternalInput")
             for i in (1, 2, 3)]

    xout_d = nc.dram_tensor("xout", [tq, d], f32, kind="ExternalOutput")
    covT_d = nc.dram_tensor("covT", [t, tq], bf16, kind="ExternalOutput")

    with tile.TileContext(nc) as tc:
        with (
            tc.tile_pool(name="consts", bufs=1) as consts,
            tc.tile_pool(name="resid", bufs=1) as residp,
            tc.tile_pool(name="hT", bufs=1) as hTp,
            tc.tile_pool(name="kv", bufs=1) as kvp,
            tc.tile_pool(name="big", bufs=1) as bigp,
            tc.tile_pool(name="ctxp", bufs=1) as ctxp,
            tc.tile_pool(name="otn", bufs=1) as otnp,
            tc.tile_pool(name="wmat", bufs=2) as wmatp,
            tc.tile_pool(name="work", bufs=2) as work,
            tc.tile_pool(name="psA", bufs=2, space="PSUM") as psA,
            tc.tile_pool(name="psB", bufs=4, space="PSUM") as psB,
        ):
            def _wdma(out, in_):
                # nowdma: timing experiment -- skip the weight transfer,
                # a 1-element memset marks the tile written
                if nowdma:
                    nc.gpsimd.memset(out[0:1, 0:1, 0:1], 0.0)
                else:
                    nc.sync.dma_start(out=out, in_=in_)

            # ---------- constants ----------
            ident = consts.tile([P, P], bf16)
            make_identity(nc, ident)
            eps_t = consts.tile([P, 1], f32)
            nc.vector.memset(eps_t, EPS)
            ones1 = consts.tile([P, P], bf16)  # row of ones at partition 64
            nc.vector.memset(ones1[64:65, :], 1.0)
            sablk_sb = None
            tri01 = None
            if flags["mask_sa"]:
                sablk_sb = consts.tile([P, 1], f32, tag="sablk")
                nc.sync.dma_start(out=sablk_sb, in_=sablk_d[:])
                # 0/1 causal keep-mask over the local triangle:
                # tri01[p, kt, q] = 1.0 where q >= kt*128 + p
                tri01 = consts.tile([P, qtt, tq], bf16, tag="tri01")
                nc.vector.memset(tri01, 1.0)
                nc.gpsimd.affine_select(
                    out=tri01, in_=tri01,
                    pattern=[[-P, qtt], [1, tq]], base=0,
                    channel_multiplier=-1,
                    compare_op=ALU.is_ge, fill=0.0)

            def opt_load(dram, flag, tag, shape):
                if not flag:
                    return None
                sb = consts.tile([P, shape], f32, tag=tag, name=tag)
                nc.sync.dma_start(out=sb, in_=dram[:])
                return sb

            bq_sb = {pre: opt_load(bq_d[pre], flags[f"bias_q_{pre}"],
                                   f"bq{pre}", dtt) for pre in ("sa", "ca")}
            bk_sb = {pre: opt_load(bk_d[pre], flags[f"bias_k_{pre}"],
                                   f"bk{pre}", dtt) for pre in ("sa", "ca")}
            b1_sb = opt_load(b1_d, flags["bias_1"], "b1", ft)
            bv_sb = {pre: opt_load(bv_d[pre], flags[f"bias_v_{pre}"],
                                   f"bv{pre}", d) for pre in ("sa", "ca")}
            bo_sb = {pre: opt_load(bo_d[pre], flags[f"bias_o_{pre}"],
                                   f"bo{pre}", d) for pre in ("sa", "ca")}
            b2_sb = opt_load(b2_d, flags["bias_2"], "b2", d)
            lng_sb = [opt_load(lng_d[i], flags[f"ln_aff{i + 1}"], f"lg{i}", d)
                      for i in range(3)]
            lnb_sb = [opt_load(lnb_d[i], flags[f"ln_aff{i + 1}"], f"lb{i}", d)
                      for i in range(3)]
            mca_sb = opt_load(mcacol_d, flags["mask_ca"], "mca", tt)

            for _rep in range(reps):
                # residual stream: own tq rows, fp32 token-major
                resid = residp.tile([P, qtt, d], f32)

                # ---------- helpers ----------
                def layernorm_to_T(order, src_ap_fn, dst_T, iln, odt=bf16,
                                   src_dt=f32, pre_fn=None, resid_dst=None):
                    """src_ap_fn(i) -> ([P, d] f32 AP, is_dram). Writes bf16
                    PE-transposed into dst_T[:, :, i*P:(i+1)*P]. pre_fn() is
                    called after the first tile's source DMA is issued (used to
                    interleave other DMA issues into the queue)."""
                    g_sb, b_sb = lng_sb[iln], lnb_sb[iln]
                    for n, i in enumerate(order):
                        src, is_dram = src_ap_fn(i)
                        if is_dram:
                            xt = work.tile([P, d], src_dt, tag="xt", bufs=4)
                            nc.sync.dma_start(out=xt, in_=src)
                            src = xt
                        if n == 0 and pre_fn is not None:
                            pre_fn()
                        st = work.tile([P, bnsub, 6], f32, tag="lnst")
                        sub = d // bnsub
                        for s in range(bnsub):
                            nc.vector.bn_stats(
                                out=st[:, s, :],
                                in_=src[:, s * sub:(s + 1) * sub])
                        mv = work.tile([P, 2], f32, tag="lnmv")
                        nc.vector.bn_aggr(out=mv, in_=st)
                        # rstd = exp(-0.5 * ln(var + eps)) -- stays in the
                        # ln/exp act table (no Sqrt table reload)
                        lnv = work.tile([P, 1], f32, tag="lnlv")
                        nc.scalar.activation(out=lnv, in_=mv[:, 1:2],
                                             func=AF.Ln, bias=eps_t, scale=1.0)
                        rstd = work.tile([P, 1], f32, tag="lnrs")
                        nc.scalar.activation(out=rstd, in_=lnv,
                                             func=AF.Exp, bias=0.0, scale=-0.5)
                        nmr = work.tile([P, 1], f32, tag="lnnm")
                        nc.vector.tensor_scalar(
                            out=nmr, in0=mv[:, 0:1], scalar1=rstd, scalar2=-1.0,
                            op0=ALU.mult, op1=ALU.mult)
                        hb = work.tile([P, d], odt, tag="lnh")
                        if g_sb is None:
                            # normalize on ScalarE: Identity(src*rstd + nmr)
                            nc.scalar.activation(
                                out=hb, in_=src, func=AF.Identity,
                                bias=nmr, scale=rstd)
                        else:
                            hf = work.tile([P, d], f32, tag="lnhf")
                            nc.vector.tensor_scalar(
                                out=hf, in0=src, scalar1=rstd, scalar2=nmr,
                                op0=ALU.mult, op1=ALU.add)
                            nc.vector.tensor_mul(out=hf, in0=hf, in1=g_sb)
                            nc.vector.tensor_add(out=hb, in0=hf, in1=b_sb)
                        for dp in range(dtt // 2):
                            tp = psB.tile([P, 2 * P], odt, tag="psB", name="tp")
                            nc.tensor.transpose(
                                tp[:, 0:P], hb[:, 2 * dp * P:(2 * dp + 1) * P],
                                ident)
                            nc.tensor.transpose(
                                tp[:, P:2 * P],
                                hb[:, (2 * dp + 1) * P:(2 * dp + 2) * P], ident)
                            nc.vector.tensor_copy(
                                out=dst_T[:, 2 * dp:2 * dp + 2, i * P:(i + 1) * P],
                                in_=tp.rearrange("p (a b) -> p a b", a=2))

                def proj_fm(dst_T, srcT, w_dram, ncols, b_sb,
                            cs_max=NCHUNK, wcs_ovr=None):
                    """Feature-major projection: dst_T [P, dtt, ncols] bf16.
                    Streams the weight in output-column chunks; cs_max trims
                    the token-chunk size (fewer source-tile deps for the
                    first matmuls), wcs_ovr the weight-chunk size (shorter
                    DMA-device occupancy per transfer)."""
                    cs = min(cs_max, ncols)
                    ncch = max(1, ncols // cs)
                    wcsl = wcs if wcs_ovr is None else wcs_ovr
                    for wc in range(d // wcsl):
                        w = wmatp.tile([P, dtt, wcsl], bf16, tag="wmat",
                                       name="wp")
                        _wdma(
                            out=w,
                            in_=w_dram[:, wc * wcsl:(wc + 1) * wcsl].rearrange(
                                "(k p) n -> p k n", p=P))
                        for mi in range(wcsl // P):
                            m = wc * (wcsl // P) + mi
                            for ch in range(ncch):
                                ps = psB.tile([P, cs], f32, tag="psB", name="pj")
                                if False:
                                    for k2 in range(dtt // 2):
                                        nc.tensor.matmul(
                                            ps,
                                            w[:, 2 * k2:2 * k2 + 2,
                                              mi * P:(mi + 1) * P],
                                            srcT[:, 2 * k2:2 * k2 + 2,
                                                 ch * cs:(ch + 1) * cs],
                                            start=(k2 == 0),
                                            stop=(k2 == dtt // 2 - 1),
                                            perf_mode=DR)
                                else:
                                    for k in range(dtt):
                                        nc.tensor.matmul(
                                            ps, w[:, k, mi * P:(mi + 1) * P],
                                            srcT[:, k, ch * cs:(ch + 1) * cs],
                                            start=(k == 0),
                                            stop=(k == dtt - 1))
                                o = dst_T[:, m, ch * cs:(ch + 1) * cs]
                                if b_sb is None:
                                    nc.scalar.copy(out=o, in_=ps)
                                else:
                                    nc.vector.tensor_scalar_add(
                                        out=o, in0=ps, scalar1=b_sb[:, m:m + 1])

                def proj_vaug(dst, srcT, wv_dram, b_bcast, ones_val=1.0,
                              csd_ovr=None):
                    """Token-major V with per-head ones column:
                    dst [P, tt, h*65] bf16; head hh at cols hh*65..hh*65+64,
                    col hh*65+64 == ones_val (softmax denominator trick).
                    For CA, V/bias/ones are pre-scaled by h so the
                    reciprocal row comes out as rb/h and coverage needs no
                    separate 1/h scaling pass."""
                    ones_ap = dst[:, :, :].rearrange(
                        "p t (hh c) -> p t hh c", c=dh + 1)[:, :, :, dh:dh + 1]
                    nc.vector.memset(ones_ap, ones_val)
                    csdl = csd if csd_ovr is None else csd_ovr
                    ncdl = d // csdl
                    hperc = csdl // dh   # heads per chunk
                    for ch in range(ncdl):
                        w = wmatp.tile([P, dtt, csdl], bf16, tag="wmat",
                                       name="wv")
                        _wdma(
                            out=w,
                            in_=wv_dram[:, ch * csdl:(ch + 1) * csdl].rearrange(
                                "(k p) n -> p k n", p=P))
                        for ti in range(tt):
                            ps = psB.tile([P, csdl], f32, tag="psB", name="pv")
                            if False:
                                for k2 in range(dtt // 2):
                                    nc.tensor.matmul(
                                        ps,
                                        srcT[:, 2 * k2:2 * k2 + 2,
                                             ti * P:(ti + 1) * P],
                                        w[:, 2 * k2:2 * k2 + 2, :],
                                        start=(k2 == 0),
                                        stop=(k2 == dtt // 2 - 1),
                                        perf_mode=DR)
                            else:
                                for k in range(dtt):
                                    nc.tensor.matmul(
                                        ps, srcT[:, k, ti * P:(ti + 1) * P],
                                        w[:, k, :],
                                        start=(k == 0), stop=(k == dtt - 1))
                            o = dst[:, ti, :].rearrange(
                                "p (hh c) -> p hh c", c=dh + 1)[
                                    :, ch * hperc:(ch + 1) * hperc, 0:dh]
                            if b_bcast is None:
                                nc.scalar.copy(
                                    out=o,
                                    in_=ps.rearrange("p (hh c) -> p hh c", c=dh))
                            else:
                                nc.vector.tensor_add(
                                    out=o,
                                    in0=ps.rearrange("p (hh c) -> p hh c", c=dh),
                                    in1=b_bcast[:, ch * csdl:(ch + 1) * csdl]
                                    .rearrange("p (hh c) -> p hh c", c=dh))

                def attention(qT, kT, vaug, sa_masked, use_mca, wo_dram,
                              bo_bcast, do_cov, covT_sb):
                    OTn = otnp.tile([P, dtt, tq], adt, tag="OTn", name="OTn")
                    atok = (None if do_cov else
                            otnp.tile([P, qtt, h, dh], bf16, tag="atok",
                                      name="atok"))
                    # coverage: bf16 pair sums held in dead est tiles;
                    # fp32 accumulation into covT alternates DVE / GpSimd
                    cov_l = [None, None]
                    cov_n = [0]

                    def cov_accum(tile_):
                        # split by key tiles: DVE and GpSimd run in parallel
                        th = tt // 2
                        nc.vector.tensor_add(
                            out=covT_sb[:, 0:th, :], in0=covT_sb[:, 0:th, :],
                            in1=tile_[:, 0:th, :])
                        nc.gpsimd.tensor_add(
                            out=covT_sb[:, th:tt, :],
                            in0=covT_sb[:, th:tt, :], in1=tile_[:, th:tt, :])
                        cov_n[0] += 1

                    def cov_push(prod):
                        if cov_l[0] is None:
                            cov_l[0] = prod
                            return
                        nc.vector.tensor_add(out=prod, in0=prod, in1=cov_l[0])
                        cov_l[0] = None
                        cov_accum(prod)

                    def cov_flush():
                        if cov_l[0] is not None:
                            cov_accum(cov_l[0])
                            cov_l[0] = None
                    def emit_scores(hh):
                        pr, sub = hh // 2, hh % 2
                        lo = 64 * sub
                        est = work.tile([P, tt, tq], bf16, tag="est", name="est",
                                        bufs=4)
                        for kp in range(kpairs):
                            sp = psA.tile([P, kper * tq], f32, tag="psA",
                                          name="sps")
                            for j in range(kper):
                                kt = kp * kper + j
                                nc.tensor.matmul(
                                    sp[:, j * tq:(j + 1) * tq],
                                    kT[lo:lo + dh, pr, kt * P:(kt + 1) * P],
                                    qT[lo:lo + dh, pr, :],
                                    start=True, stop=True, tile_position=(lo, 0))
                            spv = sp.rearrange("p (a b) -> p a b", a=kper)
                            ks = kp * kper
                            # group consecutive kt with the same exp bias
                            j0 = 0
                            while j0 < kper:
                                kt0 = ks + j0
                                if sa_masked:
                                    bias = 0.0 if kt0 < qtt else sablk_sb
                                    j1 = j0 + 1
                                    while j1 < kper and (
                                            (ks + j1 < qtt) == (kt0 < qtt)):
                                        j1 += 1
                                elif use_mca:
                                    bias = mca_sb[:, kt0:kt0 + 1]
                                    j1 = j0 + 1
                                else:
                                    bias = 0.0
                                    j1 = kper
                                nc.scalar.activation(
                                    out=est[:, ks + j0:ks + j1, :],
                                    in_=spv[:, j0:j1, :],
                                    func=AF.Exp, bias=bias, scale=1.0)
                                j0 = j1
                        if sa_masked:
                            # causal triangle over the first qtt key tiles.
                            # Stays on DVE: AV waits on this, and Pool is ~4x
                            # slower per op -- a Pool stall here blocks the
                            # in-order PE stream.
                            nc.vector.tensor_mul(
                                out=est[:, 0:qtt, :], in0=est[:, 0:qtt, :],
                                in1=tri01)
                        return est

                    def emit_post(hh, est):
                        pr, sub = hh // 2, hh % 2
                        lo = 64 * sub
                        kts = (list(range(qtt, tt)) + list(range(qtt))
                               if sa_masked and tt > qtt else list(range(tt)))
                        if not do_cov:
                            # token-major AV: out [q, head 64 + denom col].
                            # Denominators land per-partition, so the recip
                            # needs no broadcast matmul and the normalized
                            # drain needs no partition-shift DMA.
                            for qt in range(qtt):
                                avT = psB.tile([P, dh + 1], f32, tag="psB",
                                               name="avT")
                                for n_kt, kt in enumerate(kts):
                                    nc.tensor.matmul(
                                        avT,
                                        est[:, kt, qt * P:(qt + 1) * P],
                                        vaug[:, kt, hh * (dh + 1):
                                             (hh + 1) * (dh + 1)],
                                        start=(n_kt == 0),
                                        stop=(n_kt == tt - 1))
                                rc = work.tile([P, 1], f32, tag="rc",
                                               name="rc", bufs=4)
                                nc.vector.reciprocal(out=rc,
                                                     in_=avT[:, dh:dh + 1])
                                nc.vector.tensor_scalar_mul(
                                    out=atok[:, qt, hh, :],
                                    in0=avT[:, 0:dh], scalar1=rc)
                            return
                        # CA: feature-major AV with fused denominator row;
                        # the reciprocal row is broadcast to all partitions
                        # via a K=1 ones-matmul (coverage needs it anyway).
                        av = psB.tile([P, tq], f32, tag="psB", name="av")
                        for n_kt, kt in enumerate(kts):
                            nc.tensor.matmul(
                                av[0:dh + 1, :],
                                vaug[:, kt, hh * (dh + 1):(hh + 1) * (dh + 1)],
                                est[:, kt, :],
                                start=(n_kt == 0), stop=(n_kt == tt - 1))
                        rb = work.tile([P, tq], bf16, tag="rb", name="rb",
                                       bufs=2)
                        with nc.allow_low_precision(
                                reason="softmax recip row in bf16, matches "
                                       "bf16 probability envelope"):
                            nc.vector.reciprocal(out=rb[64:65, :],
                                                 in_=av[64:65, :])
                        rb_ps = psB.tile([P, tq], f32, tag="psB", name="rbps")
                        nc.tensor.matmul(rb_ps, ones1[64:65, :],
                                         rb[64:65, :], start=True,
                                         stop=True, tile_position=(64, 0))
                        nc.vector.tensor_copy(out=rb, in_=rb_ps)
                        if lo == 0:
                            nc.vector.tensor_mul(
                                out=OTn[0:dh, pr, :], in0=av[0:dh, :],
                                in1=rb[0:dh, :])
                        else:
                            otmp = work.tile([P, tq], adt, tag="otmp",
                                             name="otmp", bufs=2)
                            nc.vector.tensor_mul(
                                out=otmp[0:dh, :], in0=av[0:dh, :],
                                in1=rb[0:dh, :])
                            nc.sync.dma_start(out=OTn[lo:lo + dh, pr, :],
                                              in_=otmp[0:dh, :])
                        if do_cov:
                            # covT += est * rb: rb already equals recip/h
                            # (V, bias and the ones column are pre-scaled by
                            # h), so the bf16 broadcast multiply runs
                            # IN-PLACE on est (dead after AV) with no
                            # separate 1/h pass; partial sums tree up
                            # through the est ring.
                            rb_b = rb[:].rearrange(
                                "p (a q) -> p a q", a=1).broadcast_to(
                                [P, tt, tq])
                            if hh == 0:
                                nc.vector.tensor_mul(
                                    out=covT_sb, in0=est, in1=rb_b)
                            else:
                                nc.vector.tensor_mul(
                                    out=est, in0=est, in1=rb_b)
                                cov_push(est)

                    # software-pipelined head loop: head h's scores hide the
                    # exp->AV latency of head h-1
                    prev = None
                    for hh in range(h):
                        est = emit_scores(hh)
                        if prev is not None:
                            emit_post(*prev)
                        prev = (hh, est)
                    emit_post(*prev)
                    if do_cov:
                        cov_flush()
                    else:
                        # transpose token-major attention output into the
                        # feature-major OTn the O-projection consumes
                        for qt in range(qtt):
                            for j2 in range(dtt // 2):
                                tp = psB.tile([P, 2 * P], bf16, tag="psB",
                                              name="tpo")
                                nc.tensor.transpose(
                                    tp[:, 0:P],
                                    atok[:, qt, 4 * j2:4 * j2 + 2, :]
                                    .rearrange("p a b -> p (a b)"), ident)
                                nc.tensor.transpose(
                                    tp[:, P:2 * P],
                                    atok[:, qt, 4 * j2 + 2:4 * j2 + 4, :]
                                    .rearrange("p a b -> p (a b)"), ident)
                                nc.vector.tensor_copy(
                                    out=OTn[:, 2 * j2:2 * j2 + 2,
                                            qt * P:(qt + 1) * P],
                                    in_=tp.rearrange("p (a b) -> p a b", a=2))
                    # output projection + residual. Both weight chunks stay
                    # resident so the qt loop runs outermost -- resid[:, qt]
                    # finalizes per query tile, letting the next layernorm
                    # start while later tiles still project.
                    wos = []
                    for wc in range(ncd):
                        w = wmatp.tile([P, dtt, csd], adt, tag="wmat", name="wo")
                        _wdma(
                            out=w,
                            in_=wo_dram[:, wc * csd:(wc + 1) * csd].rearrange(
                                "(k p) n -> p k n", p=P))
                        wos.append(w)
                    for qt in range(qtt):
                        for wc in range(ncd):
                            op = psB.tile([P, csd], f32, tag="psB", name="op")
                            if dr:
                                for k2 in range(dtt // 2):
                                    nc.tensor.matmul(
                                        op,
                                        OTn[:, 2 * k2:2 * k2 + 2,
                                            qt * P:(qt + 1) * P],
                                        wos[wc][:, 2 * k2:2 * k2 + 2, :],
                                        start=(k2 == 0),
                                        stop=(k2 == dtt // 2 - 1),
                                        perf_mode=DR)
                            else:
                                for k in range(dtt):
                                    nc.tensor.matmul(
                                        op, OTn[:, k, qt * P:(qt + 1) * P],
                                        wos[wc][:, k, :],
                                        start=(k == 0), stop=(k == dtt - 1))
                            rslice = resid[:, qt, wc * csd:(wc + 1) * csd]
                            if bo_bcast is not None:
                                nc.vector.tensor_add(
                                    out=op, in0=op,
                                    in1=bo_bcast[:, wc * csd:(wc + 1) * csd])
                            nc.vector.tensor_add(out=rslice, in0=rslice, in1=op)
                    if do_cov:
                        nc.sync.dma_start(
                            out=covT_d[:].rearrange("(k p) q -> p k q", p=P),
                            in_=covT_sb)

                # ---------- phase 1: LN1 over full sequence -> h1T ----------
                # Every LN1 tile streams from DRAM in bf16; the own tiles
                # (0..qtt-1) are widened to fp32 on GpSimd to seed the
                # residual stream (no separate x load). The Q projection is
                # emitted between the two LN1 halves: it only needs h1T
                # columns 0..tq, and emitting it early keeps its PSUM-ring
                # slots ahead of the second half's transposes (ring slots
                # hand out in emission order).
                def issue_resid_load():
                    nc.sync.dma_start(
                        out=resid,
                        in_=x_d[0:tq, :].rearrange("(q p) d -> p q d", p=P))

                h1T = hTp.tile([P, dtt, t], bf16, tag="hT", name="h1T")
                lnsrc = lambda i: (xb_d[i * P:(i + 1) * P, :], True)
                layernorm_to_T(range(qtt), lnsrc, h1T, 0, src_dt=bf16)

                # ---------- phase 2: self-attention ----------
                qT = kvp.tile([P, dtt, tq], bf16, tag="qT", name="qT1")
                proj_fm(qT, h1T[:, :, 0:tq], w_d["sa_q"], tq, bq_sb["sa"],
                        cs_max=2 * P, wcs_ovr=2 * P)
                layernorm_to_T(range(qtt, tt), lnsrc, h1T, 0, src_dt=bf16)
                kT = kvp.tile([P, dtt, t], bf16, tag="kT", name="kT1")
                proj_fm(kT, h1T, w_d["sa_k"], t, bk_sb["sa"])
                vaug = kvp.tile([P, tt, h * (dh + 1)], bf16, tag="vaug",
                                name="va1")
                proj_vaug(vaug, h1T, w_d["sa_v"], bv_sb["sa"])
                issue_resid_load()
                attention(qT, kT, vaug, flags["mask_sa"], False, w_d["sa_o"],
                          bo_sb["sa"], False, None)

                # ---------- phase 3: LN2 -> h2T, cross-attention ----------
                ctxT = ctxp.tile([P, dtt, t], bf16, tag="ctxT")
                nc.sync.dma_start(
                    out=ctxT, in_=ctxT_d[:].rearrange("(k p) t -> p k t", p=P))
                h2T = hTp.tile([P, dtt, tq], bf16, tag="hT", name="h2T")
                layernorm_to_T(range(qtt), lambda i: (resid[:, i, :], False),
                               h2T, 1)
                qT2 = kvp.tile([P, dtt, tq], bf16, tag="qT", name="qT2")
                proj_fm(qT2, h2T, w_d["ca_q"], tq, bq_sb["ca"])
                kT2 = kvp.tile([P, dtt, t], bf16, tag="kT", name="kT2")
                proj_fm(kT2, ctxT, w_d["ca_k"], t, bk_sb["ca"])
                vaug2 = kvp.tile([P, tt, h * (dh + 1)], bf16, tag="vaug",
                                 name="va2")
                proj_vaug(vaug2, ctxT, w_d["ca_v"], bv_sb["ca"],
                          ones_val=float(h))
                # bf16 coverage accumulator: halves SBUF and doubles DVE
                # add throughput; ~0.3% extra rounding on cov (within tol)
                covT_sb = bigp.tile([P, tt, tq], bf16, tag="big",
                                    name="covT_sb")
                attention(qT2, kT2, vaug2, False, flags["mask_ca"], w_d["ca_o"],
                          bo_sb["ca"], True, covT_sb)

                # ---------- phase 4: LN3 -> h3T, FFN ----------
                h3T = hTp.tile([P, dtt, tq], adt, tag="hT", name="h3T")
                layernorm_to_T(range(qtt), lambda i: (resid[:, i, :], False),
                               h3T, 2, odt=adt)
                # fT reuses the ctxT ring (dead after the CA K/V projections)
                # rather than covT's -- covT is still accumulating when FF1's
                # first drains land
                fT = ctxp.tile([P, ft, tq], adt, tag="ctxT", name="fT")
                for fc in range(f // csf):
                    w1c = wmatp.tile([P, dtt, csf], adt, tag="wmat", name="w1c")
                    _wdma(
                        out=w1c,
                        in_=w1_d[:, fc * csf:(fc + 1) * csf].rearrange(
                            "(k p) n -> p k n", p=P))
                    # two f-tiles per PSUM tile (psA-sized) -- halves the Relu
                    # drain count and leaves psB free for the FF2 accumulators
                    # so FF1 and FF2 can interleave
                    for fm2 in range(csf // (2 * P)):
                        fi = fc * (csf // P) + 2 * fm2
                        ps = psA.tile([P, 2, tq], f32, tag="psA", name="pf")
                        for half in range(2):
                            if dr:
                                for k2 in range(dtt // 2):
                                    nc.tensor.matmul(
                                        ps[:, half, :],
                                        w1c[:, 2 * k2:2 * k2 + 2,
                                            (2 * fm2 + half) * P:
                                            (2 * fm2 + half + 1) * P],
                                        h3T[:, 2 * k2:2 * k2 + 2, :],
                                        start=(k2 == 0),
                                        stop=(k2 == dtt // 2 - 1),
                                        perf_mode=DR)
                            else:
                                for k in range(dtt):
                                    nc.tensor.matmul(
                                        ps[:, half, :],
                                        w1c[:, k, (2 * fm2 + half) * P:
                                            (2 * fm2 + half + 1) * P],
                                        h3T[:, k, :],
                                        start=(k == 0), stop=(k == dtt - 1))
                        if b1_sb is None:
                            nc.scalar.activation(
                                out=fT[:, fi:fi + 2, :], in_=ps,
                                func=AF.Relu, bias=0.0, scale=1.0)
                        else:
                            for half in range(2):
                                nc.scalar.activation(
                                    out=fT[:, fi + half, :], in_=ps[:, half, :],
                                    func=AF.Relu,
                                    bias=b1_sb[:, fi + half:fi + half + 1],
                                    scale=1.0)
                # FF2: single pass over w2, one PSUM accumulator per query tile
                kchunks = ft // dtt
                for dc in range(ncd):
                    accs = [psB.tile([P, csd], f32, tag="psB", name=f"ac{qt}")
                            for qt in range(qtt)]
                    for kc in range(kchunks):
                        w2c = wmatp.tile([P, dtt, csd], adt, tag="wmat",
                                         name="w2c")
                        _wdma(
                            out=w2c,
                            in_=w2_d[kc * dtt * P:(kc + 1) * dtt * P,
                                     dc * csd:(dc + 1) * csd].rearrange(
                                "(k p) n -> p k n", p=P))
                        for qt in range(qtt):
                            if dr:
                                for k2 in range(dtt // 2):
                                    kk2 = kc * (dtt // 2) + k2
                                    nc.tensor.matmul(
                                        accs[qt],
                                        fT[:, 2 * kk2:2 * kk2 + 2,
                                           qt * P:(qt + 1) * P],
                                        w2c[:, 2 * k2:2 * k2 + 2, :],
                                        start=(kk2 == 0),
                                        stop=(kk2 == ft // 2 - 1),
                                        perf_mode=DR)
                            else:
                                for k in range(dtt):
                                    kk = kc * dtt + k
                                    nc.tensor.matmul(
                                        accs[qt],
                                        fT[:, kk, qt * P:(qt + 1) * P],
                                        w2c[:, k, :],
                                        start=(kk == 0), stop=(kk == ft - 1))
                    for qt in range(qtt):
                        rslice = resid[:, qt, dc * csd:(dc + 1) * csd]
                        if b2_sb is not None:
                            nc.vector.tensor_add(
                                out=accs[qt], in0=accs[qt],
                                in1=b2_sb[:, dc * csd:(dc + 1) * csd])
                        nc.vector.tensor_add(out=rslice, in0=rslice,
                                             in1=accs[qt])
                        nc.sync.dma_start(
                            out=xout_d[qt * P:(qt + 1) * P,
                                       dc * csd:(dc + 1) * csd],
                            in_=rslice)
    nc.finalize()   # Bacc legalization (reg alloc, event-semaphore splits)
    _dedup_act_loads(nc)
    return nc


def _prep_inputs(x, context, mask_tgt, mask_src, weights, t, tq, d, h, f):
    """Build per-core in_maps. Returns (in_maps, flags, perms)."""
    ft, dtt, tt = f // P, d // P, t // P
    b = x.shape[0]
    qsh = t // tq
    dh = d // h

    def bf(a):
        return np.ascontiguousarray(a.astype(ml_dtypes.bfloat16))

    dr = FP8 and dtt % 2 == 0
    wnp = mybir.dt.np(fp8) if dr else ml_dtypes.bfloat16

    def wq8(a):
        return np.ascontiguousarray(a.astype(np.float32).astype(wnp))

    def f32c(a):
        return np.ascontiguousarray(a.astype(np.float32))

    def pp(vec, ntiles, scale=1.0):
        return f32c((vec.astype(np.float32) * scale).reshape(ntiles, P).T)

    def bcast(vec):
        return f32c(np.broadcast_to(vec.astype(np.float32)[None, :], (P, d)))

    flags = {
        "mask_sa": bool(mask_tgt.any()),
        "mask_ca": bool(mask_src.any()),
        "bias_q_sa": bool(np.any(weights["sa_bq"])),
        "bias_q_ca": bool(np.any(weights["ca_bq"])),
        "bias_k_sa": bool(np.any(weights["sa_bk"])),
        "bias_k_ca": bool(np.any(weights["ca_bk"])),
        "bias_v_sa": bool(np.any(weights["sa_bv"])),
        "bias_v_ca": bool(np.any(weights["ca_bv"])),
        "bias_o_sa": bool(np.any(weights["sa_bo"])),
        "bias_o_ca": bool(np.any(weights["ca_bo"])),
        "bias_1": bool(np.any(weights["ff_b1"])),
        "bias_2": bool(np.any(weights["ff_b2"])),
        "ln_aff1": not (np.all(weights["ln1_g"] == 1)
                        and not np.any(weights["ln1_b"])),
        "ln_aff2": not (np.all(weights["ln2_g"] == 1)
                        and not np.any(weights["ln2_b"])),
        "ln_aff3": not (np.all(weights["ln3_g"] == 1)
                        and not np.any(weights["ln3_b"])),
    }

    sc = dh ** -0.5
    shared = {
        "wq_sa": bf(weights["sa_Wq"] * sc), "wk_sa": bf(weights["sa_Wk"]),
        "wv_sa": bf(weights["sa_Wv"]), "wo_sa": wq8(weights["sa_Wo"]),
        "wq_ca": bf(weights["ca_Wq"] * sc), "wk_ca": bf(weights["ca_Wk"]),
        # CA V scaled by h (exact power of 2): the AV denominator row comes
        # out h-scaled, so its reciprocal is directly rb/h for coverage
        "wv_ca": bf(weights["ca_Wv"] * h), "wo_ca": wq8(weights["ca_Wo"]),
        "w1": wq8(weights["ff_W1"]), "w2": wq8(weights["ff_W2"]),
        "bq_sa": pp(weights["sa_bq"], dtt, sc),
        "bq_ca": pp(weights["ca_bq"], dtt, sc),
        "bk_sa": pp(weights["sa_bk"], dtt),
        "bk_ca": pp(weights["ca_bk"], dtt),
        "b1": pp(weights["ff_b1"], ft),
        "bv_sa": bcast(weights["sa_bv"]), "bv_ca": bcast(weights["ca_bv"] * h),
        "bo_sa": bcast(weights["sa_bo"]), "bo_ca": bcast(weights["ca_bo"]),
        "b2": bcast(weights["ff_b2"]),
    }
    for nm in ("ln1", "ln2", "ln3"):
        shared[f"{nm}_g"] = bcast(weights[f"{nm}_g"])
        shared[f"{nm}_b"] = bcast(weights[f"{nm}_b"])

    mt = np.broadcast_to(mask_tgt, (b, t, t))
    ms = np.broadcast_to(mask_src.reshape(b, -1), (b, t))

    in_maps, perms = [], []
    for c in range(b * qsh):
        bi, qh = c // qsh, c % qsh
        qs = qh * tq
        perm = np.concatenate([np.arange(qs, qs + tq),
                               np.arange(0, qs),
                               np.arange(qs + tq, t)]).astype(np.int64)
        perms.append(perm)
        # permuted-order mask structure: local causal triangle over the
        # first tq keys + an all-or-nothing block over the rest
        mp = mt[bi][qs:qs + tq][:, perm]
        blk = NEGMASK if (t > tq and bool(mp[:, tq:].all())) else 0.0
        # mask_ca as per-k column bias [P, tt] (k = kt*P + p)
        mca = (ms[bi].astype(np.float32) * NEGMASK).reshape(tt, P).T
        im = dict(shared)
        im["x"] = f32c(x[bi][perm])
        im["xb"] = bf(x[bi][perm])
        im["ctxT"] = bf(context[bi].T)
        im["sa_blk"] = np.full((P, 1), blk, np.float32)
        im["mca_col"] = f32c(mca)
        in_maps.append(im)
    return in_maps, flags, perms


_CACHE = {}


def _get_program(key, t, tq, d, h, f, flags):
    ck = (key, tuple(sorted(flags.items())))
    if ck not in _CACHE:
        _CACHE[ck] = _build_program(t, tq, d, h, f, flags)
    return _CACHE[ck]


def kernel(x, context, mask_tgt, mask_src,
           ln1_g, ln1_b, ln2_g, ln2_b, ln3_g, ln3_b,
           sa_Wq, sa_bq, sa_Wk, sa_bk, sa_Wv, sa_bv, sa_Wo, sa_bo,
           ca_Wq, ca_bq, ca_Wk, ca_bk, ca_Wv, ca_bv, ca_Wo, ca_bo,
           ff_W1, ff_b1, ff_W2, ff_b2, _run=None):
    x = np.asarray(x, np.float32)
    context = np.asarray(context, np.float32)
    mask_tgt = np.asarray(mask_tgt, bool)
    mask_src = np.asarray(mask_src, bool)
    weights = {k: np.asarray(v) for k, v in dict(
        ln1_g=ln1_g, ln1_b=ln1_b, ln2_g=ln2_g, ln2_b=ln2_b,
        ln3_g=ln3_g, ln3_b=ln3_b,
        sa_Wq=sa_Wq, sa_bq=sa_bq, sa_Wk=sa_Wk, sa_bk=sa_bk,
        sa_Wv=sa_Wv, sa_bv=sa_bv, sa_Wo=sa_Wo, sa_bo=sa_bo,
        ca_Wq=ca_Wq, ca_bq=ca_bq, ca_Wk=ca_Wk, ca_bk=ca_bk,
        ca_Wv=ca_Wv, ca_bv=ca_bv, ca_Wo=ca_Wo, ca_bo=ca_bo,
        ff_W1=ff_W1, ff_b1=ff_b1, ff_W2=ff_W2, ff_b2=ff_b2).items()}

    t, tq, d, h, f = T, TQ, D, H, F
    in_maps, flags, perms = _prep_inputs(
        x, context, mask_tgt, mask_src, weights, t, tq, d, h, f)
    nc = _get_program("full", t, tq, d, h, f, flags)
    if _run is None:
        res = run_bass_kernel_spmd(nc, in_maps,
                                   core_ids=list(range(len(in_maps)))).results
    else:
        res = _run(nc, in_maps)

    b, qsh = x.shape[0], t // tq
    xout = np.empty((b, t, d), np.float32)
    cov = np.empty((b, t, t), np.float32)
    for c, r in enumerate(res):
        bi, qh = c // qsh, c % qsh
        qs = qh * tq
        xout[bi, qs:qs + tq] = r["xout"]
        cov[bi, qs:qs + tq] = r["covT"].T
    return xout, cov

